# revision 3
# baseline (speedup 1.0000x reference)
"""ConvDownsample2D (StyleGAN2 FIR blur + strided conv) for 8 Trainium2 cores.

Strategy (data-parallel over batch, 1 image per core):
  - The separable FIR blur k2d = outer(k,k)/sum^2 is split into an H-pass and
    a V-pass.
  - H-pass is done ON THE TENSOR ENGINE as a banded matmul  y_h = x_wtile.T @ B
    (contraction over image columns), which SIMULTANEOUSLY transposes the
    NHWC input into channel-major layout for free.
  - V-pass runs on the vector engine as 4 scalar_tensor_tensor taps (fp16).
  - The 3x3/stride-2 conv is 9 accumulating matmuls per output row
    (lhsT = blurred activations [C,128pix], rhs = W taps [C,256oc]) plus a
    K=1 matmul that adds the bias; accumulation in fp32 PSUM.
  - Inputs are fed to the device in fp16 (host-side cast): full-rate PE at any
    free dim, half the DMA traffic. fp32 accumulation keeps rel err ~4e-4.
"""
import sys

if "/opt/trn_rl_repo" not in sys.path:
    sys.path.insert(0, "/opt/trn_rl_repo")

import numpy as np

import concourse.bass as bass
import concourse.tile as tile
from concourse import bacc, mybir
from concourse.bass_utils import run_bass_kernel_spmd

F16 = mybir.dt.float16
F32 = mybir.dt.float32

N_CORES = 8
H = W = 256
C = 128
OC = 256
OH = OW = 128
WP = W + 1          # 257 blurred width
PITCH = 258         # even row pitch for fp16 4B alignment
P_BLK = 32          # output rows per block
N_BLK = OH // P_BLK


def _build_bass():
    nc = bacc.Bacc("TRN2", target_bir_lowering=False, debug=False)

    x16 = nc.dram_tensor("x16", [H, W, C], F16, kind="ExternalInput").ap()
    b_a = nc.dram_tensor("b_a", [128, 131], F16, kind="ExternalInput").ap()
    b_b = nc.dram_tensor("b_b", [128, 130], F16, kind="ExternalInput").ap()
    w16 = nc.dram_tensor("w16", [9, C, OC], F16, kind="ExternalInput").ap()
    kvt = nc.dram_tensor("kvt", [128, 4], F32, kind="ExternalInput").ap()
    ones = nc.dram_tensor("ones", [1, 128], F16, kind="ExternalInput").ap()
    bias = nc.dram_tensor("bias", [1, OC], F16, kind="ExternalInput").ap()
    out = nc.dram_tensor("out", [OH, OW, OC], F32, kind="ExternalOutput").ap()

    with tile.TileContext(nc) as tc:
        with (
            tc.tile_pool(name="const", bufs=1) as cpool,
            tc.tile_pool(name="xin", bufs=8) as xpool,
            tc.tile_pool(name="yh", bufs=2) as yhpool,
            tc.tile_pool(name="yv", bufs=2) as yvpool,
            tc.tile_pool(name="osb", bufs=4) as opool,
            tc.tile_pool(name="pyh", bufs=3, space=bass.MemorySpace.PSUM) as pyh,
            tc.tile_pool(name="pout", bufs=2, space=bass.MemorySpace.PSUM) as pout,
        ):
            ba_sb = cpool.tile([128, 131], F16)
            nc.sync.dma_start(ba_sb[:], b_a[:])
            bb_sb = cpool.tile([128, 130], F16)
            nc.sync.dma_start(bb_sb[:], b_b[:])
            w_sb = cpool.tile([128, 9, OC], F16)
            for t in range(9):
                nc.sync.dma_start(w_sb[:, t, :], w16[t])
            kv_sb = cpool.tile([128, 4], F32)
            nc.sync.dma_start(kv_sb[:], kvt[:])
            ones_sb = cpool.tile([1, 128], F16)
            nc.sync.dma_start(ones_sb[:], ones[:])
            bias_sb = cpool.tile([1, OC], F16)
            nc.sync.dma_start(bias_sb[:], bias[:])

            for k in range(N_BLK):
                p0 = k * P_BLK
                hs0 = 2 * p0 - 2          # first y_h row (may be <0)
                n_yh = 2 * P_BLK + 4      # 68 y_h rows incl halo
                n_yv = 2 * P_BLK + 1      # 65 y_v rows

                yh_t = yhpool.tile([128, n_yh, PITCH], F16)
                # zero-pad rows outside the image (always aligned to pairs)
                if hs0 < 0:
                    nc.gpsimd.memset(yh_t[:, 0:2, :], 0.0)
                if hs0 + n_yh > H:
                    nc.gpsimd.memset(yh_t[:, n_yh - 2 : n_yh, :], 0.0)

                # ---- H-blur (+ transpose) on PE ----
                for s0 in range(0, n_yh, 2):
                    if hs0 + s0 < 0 or hs0 + s0 + 1 >= H:
                        continue
                    pp = pyh.tile([128, 2, 512], F32)
                    for e in range(2):
                        h = hs0 + s0 + e
                        xlo = xpool.tile([128, C], F16)
                        nc.sync.dma_start(xlo[:], x16[h, 0:128, :])
                        xhi = xpool.tile([128, C], F16)
                        nc.sync.dma_start(xhi[:], x16[h, 128:256, :])
                        nc.tensor.matmul(
                            pp[:, e, 0:131], xlo[:], ba_sb[:],
                            start=True, stop=False,
                        )
                        nc.tensor.matmul(
                            pp[:, e, 127:257], xhi[:], bb_sb[:],
                            start=False, stop=True, skip_group_check=True,
                        )
                    nc.scalar.copy(yh_t[:, s0 : s0 + 2, 0:WP], pp[:, :, 0:WP])

                # ---- V-blur on DVE (4 taps, in-place accumulate) ----
                yv_t = yvpool.tile([128, n_yv, PITCH], F16)
                nc.vector.tensor_scalar(
                    yv_t[:, :, :],
                    yh_t[:, 0:n_yv, :],
                    kv_sb[:, 0:1],
                    None,
                    mybir.AluOpType.mult,
                )
                for u in range(1, 4):
                    nc.vector.scalar_tensor_tensor(
                        yv_t[:, :, :],
                        yh_t[:, u : u + n_yv, :],
                        kv_sb[:, u : u + 1],
                        yv_t[:, :, :],
                        mybir.AluOpType.mult,
                        mybir.AluOpType.add,
                    )

                # ---- conv 3x3 stride 2 + bias on PE ----
                for pr in range(P_BLK // 2):
                    po = pout.tile([128, 2, OC], F32)  # one bank: 2 out rows
                    for e in range(2):
                        p = p0 + 2 * pr + e
                        r0 = 2 * (p - p0)  # y_v slot of first tap row
                        for t in range(9):
                            a, b = divmod(t, 3)
                            lhsT = yv_t[:, r0 + a, b : b + 256 : 2]
                            nc.tensor.matmul(
                                po[:, e, :], lhsT, w_sb[:, t, :],
                                start=(e == 0 and t == 0), stop=False,
                                skip_group_check=True,
                            )
                        nc.tensor.matmul(
                            po[:, e, :], ones_sb[:], bias_sb[:],
                            start=False, stop=(e == 1),
                            skip_group_check=True,
                        )
                    ot = opool.tile([128, 2, OC], F32)
                    if pr % 2 == 0:
                        nc.vector.tensor_copy(ot[:], po[:])
                    else:
                        nc.scalar.copy(ot[:], po[:])
                    p = p0 + 2 * pr
                    dst = out[p : p + 2, :, :].rearrange("r q o -> q r o")
                    nc.sync.dma_start(dst, ot[:])

    nc.compile()
    return nc


_NC = None


def _get_nc():
    global _NC
    if _NC is None:
        _NC = _build_bass()
    return _NC


def _prepare_in_maps(x, conv_w, conv_b, blur_kernel):
    x = np.asarray(x, dtype=np.float32)
    conv_w = np.asarray(conv_w, dtype=np.float32)
    conv_b = np.asarray(conv_b, dtype=np.float32)
    bk = np.asarray(blur_kernel, dtype=np.float32)

    # host-side prep of tiny derived tensors
    k1 = (bk / bk.sum()).astype(np.float32)  # separable normalized taps
    Bfull = np.zeros((W, WP), np.float32)
    j = np.arange(W)[:, None]
    wp = np.arange(WP)[None, :]
    d = j - wp + 2
    mask = (d >= 0) & (d <= 3)
    Bfull[mask] = k1[d[mask]]
    b_a = Bfull[0:128, 0:131].astype(np.float16)
    b_b = Bfull[128:256, 127:257].astype(np.float16)
    w16 = np.ascontiguousarray(
        conv_w.reshape(9, C, OC).astype(np.float16)
    )
    kvt = np.broadcast_to(k1[None, :], (128, 4)).astype(np.float32)
    kvt = np.ascontiguousarray(kvt)
    ones = np.ones((1, 128), np.float16)
    bias = conv_b.reshape(1, OC).astype(np.float16)

    in_maps = []
    for i in range(N_CORES):
        in_maps.append(
            {
                "x16": np.ascontiguousarray(x[i].astype(np.float16)),
                "b_a": b_a,
                "b_b": b_b,
                "w16": w16,
                "kvt": kvt,
                "ones": ones,
                "bias": bias,
            }
        )
    return in_maps


def _run(in_maps, **kwargs):
    nc = _get_nc()
    return run_bass_kernel_spmd(nc, in_maps, core_ids=list(range(N_CORES)), **kwargs)


def kernel(x, conv_w, conv_b, blur_kernel):
    in_maps = _prepare_in_maps(x, conv_w, conv_b, blur_kernel)
    res = _run(in_maps)
    return np.stack([res.results[i]["out"] for i in range(N_CORES)], axis=0)
